# revision 18
# baseline (speedup 1.0000x reference)
"""Expert-parallel MoE FFN kernel for Trainium2 (8 NeuronCores).

Problem: y[e] = relu(x[e] @ w1[e].T) @ w2[e].T for 8 experts.
Sharding: expert-parallel — expert e runs entirely on core e; no
cross-core communication.

Host-side prep (part of the sharding step): each expert's x, w1, w2 are
transposed to the layouts the PE array consumes (contraction dim on
partitions) and cast to bf16. That removes all 640 on-device PE
transposes (which cost ~120us of tensor-engine time via unhidden
LDWEIGHTS) and cuts DMA-in from 40MB fp32 to 20MB bf16.

Per-core dataflow (xT:[1024,2048], w1T:[1024,4096], w2T:[4096,1024]):
  w1T and w2T live in SBUF for the whole kernel (64 KiB/partition each).
  For each 512-token block:
    GEMM1: hT[dh_tile, tok] accumulated in PSUM over the 8 d_model
           tiles; fused ReLU + bf16 cast on eviction into a resident
           hT[128, 32, 512] tile. No DRAM spill.
    GEMM2: y[tok_tile, dm] accumulated in PSUM over the 32 d_hidden
           tiles against resident w2T; evicted fp32 and DMA'd out.
  All matmuls stream 512 columns (213ns) which hides every LDWEIGHTS;
  tensor-engine work is the 2048 essential GEMM matmuls only.
"""

import sys

if "/opt/trn_rl_repo" not in sys.path:
    sys.path.insert(0, "/opt/trn_rl_repo")

import ml_dtypes
import numpy as np

import concourse.bass as bass  # noqa: F401
import concourse.mybir as mybir
from concourse import bacc
from concourse.bass_utils import run_bass_kernel_spmd
from concourse.tile import TileContext

P = 128
TOK = 2048
DM = 1024
DH = 4096
N_CORES = 8

KD = DM // P  # 8 d_model tiles (GEMM1 contraction)
KH = DH // P  # 32 d_hidden tiles (GEMM2 contraction)
TB = 512  # token block
NTB = TOK // TB  # 4 token blocks
MPB = TB // P  # 4 token tiles per block
QW = DM  # w1 load chunk (dh columns per DMA)

BF16 = ml_dtypes.bfloat16


def build_nc():
    f32 = mybir.dt.float32
    bf = mybir.dt.bfloat16
    nc = bacc.Bacc("TRN2", target_bir_lowering=False, debug=False)
    xT = nc.dram_tensor("xT", [DM, TOK], bf, kind="ExternalInput")
    w1T = nc.dram_tensor("w1T", [DM, DH], bf, kind="ExternalInput")
    w2T = nc.dram_tensor("w2T", [DH, DM], bf, kind="ExternalInput")
    y = nc.dram_tensor("y", [TOK, DM], f32, kind="ExternalOutput")

    relu = mybir.ActivationFunctionType.Relu
    copyf = mybir.ActivationFunctionType.Copy

    with TileContext(nc) as tc:
        with (
            tc.tile_pool(name="w1res", bufs=1) as w1p,
            tc.tile_pool(name="w2res", bufs=1) as w2p,
            tc.tile_pool(name="xt", bufs=NTB) as xp,
            tc.tile_pool(name="ht", bufs=1) as hp,
            tc.tile_pool(name="ys", bufs=4) as yp,
            tc.tile_pool(name="mm", bufs=8, space="PSUM") as mmp,
        ):
            w1t = w1p.tile([P, KD, DH], bf, name="w1t")
            w2t = w2p.tile([P, KH, DM], bf, name="w2t")

            # Startup: interleave first x block and first w1 chunk per kt
            # so GEMM1's first accumulation group can start as soon as the
            # kt=0 pair lands (DMA issue on the sync engine costs ~630ns
            # per dma_start, so issue order is arrival order).
            # All DMAs issue from the sync engine: its dma_start is the
            # hardware-queue path. (gpsimd.dma_start turns into software
            # dynamic DMA, which steals SBUF bandwidth from the PE and
            # slows every matmul ~20%.) Issue order is consumption order;
            # the first x block and first w1 chunk interleave per kt so
            # GEMM1's first group starts as soon as the kt=0 pair lands.
            xb = [xp.tile([P, KD, TB], bf, name="xt") for _ in range(NTB)]
            for kt in range(KD):
                if kt == 0:
                    # halve the very first pair so the first matmul's
                    # operands land ~1us sooner
                    nc.sync.dma_start(
                        xb[0][:, 0, 0 : TB // 2], xT[0:P, 0 : TB // 2]
                    )
                    nc.sync.dma_start(
                        w1t[:, 0, 0 : QW // 2], w1T[0:P, 0 : QW // 2]
                    )
                    nc.sync.dma_start(
                        xb[0][:, 0, TB // 2 : TB], xT[0:P, TB // 2 : TB]
                    )
                    nc.sync.dma_start(
                        w1t[:, 0, QW // 2 : QW], w1T[0:P, QW // 2 : QW]
                    )
                    continue
                nc.sync.dma_start(
                    xb[0][:, kt, :], xT[kt * P : (kt + 1) * P, 0:TB]
                )
                nc.sync.dma_start(
                    w1t[:, kt, 0:QW], w1T[kt * P : (kt + 1) * P, 0:QW]
                )
            for q in range(1, DH // QW):
                for kt in range(KD):
                    nc.sync.dma_start(
                        w1t[:, kt, q * QW : (q + 1) * QW],
                        w1T[kt * P : (kt + 1) * P, q * QW : (q + 1) * QW],
                    )
            for ht in range(KH):
                nc.sync.dma_start(w2t[:, ht, :], w2T[ht * P : (ht + 1) * P, :])
            for tb in range(1, NTB):
                for kt in range(KD):
                    nc.sync.dma_start(
                        xb[tb][:, kt, :],
                        xT[kt * P : (kt + 1) * P, tb * TB : (tb + 1) * TB],
                    )

            # PE warmup: dummy matmuls with no DMA dependency fill the
            # ~6us window while the first x/w1 chunks land, ramping the
            # tensor engine out of its low p-state before real work.
            wu = w1p.tile([P, TB], bf, name="wu")
            nc.vector.memset(wu[:], 0.0)
            for _ in range(8):
                pw = mmp.tile([P, TB], f32, tag="ps", name="psw")
                nc.tensor.matmul(pw[:], wu[:, 0:P], wu[:], start=True, stop=True)

            ncopy = [0]  # alternate PSUM->SBUF eviction engine

            def evict(dst, src, do_relu):
                ncopy[0] += 1
                if ncopy[0] % 2 == 0:
                    if do_relu:
                        nc.vector.tensor_scalar_max(dst, src, 0.0)
                    else:
                        nc.vector.tensor_copy(dst, src)
                else:
                    nc.scalar.activation(dst, src, relu if do_relu else copyf)

            for tb in range(NTB):
                hT = hp.tile([P, KH, TB], bf, name="hT")
                # GEMM1: hT[ht, tok] = relu(sum_kt w1T[kt,ht].T @ xT[kt,tok])
                for ht in range(KH):
                    ps = mmp.tile([P, TB], f32, tag="ps", name="ps1")
                    for kt in range(KD):
                        nc.tensor.matmul(
                            ps[:],
                            w1t[:, kt, ht * P : (ht + 1) * P],
                            xb[tb][:, kt, :],
                            start=(kt == 0),
                            stop=(kt == KD - 1),
                        )
                    evict(hT[:, ht, :], ps[:], True)
                # GEMM2: y[mt, dm] = sum_ht hT[ht, mt].T @ w2T[ht, dm]
                for mt in range(MPB):
                    for db in range(2):
                        # split the kernel's very last group in two so the
                        # final evict+DMA tail after the last matmul is half
                        # as long (matmul out is capped at 512 = 1 PSUM bank)
                        last = tb == NTB - 1 and mt == MPB - 1 and db == 1
                        for c0, cw in ((0, 256), (256, 256)) if last else ((0, TB),):
                            ps = mmp.tile([P, cw], f32, tag="ps", name="ps2")
                            col = db * TB + c0
                            for ht in range(KH):
                                nc.tensor.matmul(
                                    ps[:],
                                    hT[:, ht, mt * P : (mt + 1) * P],
                                    w2t[:, ht, col : col + cw],
                                    start=(ht == 0),
                                    stop=(ht == KH - 1),
                                )
                            ys = yp.tile([P, cw], f32, name="ys")
                            evict(ys[:], ps[:], False)
                            row = tb * TB + mt * P
                            nc.sync.dma_start(y[row : row + P, col : col + cw], ys[:])
    nc.compile()
    return nc


def make_in_maps(x, weight1, weight2):
    return [
        {
            "xT": x[e].T.astype(BF16),
            "w1T": weight1[e].T.astype(BF16),
            "w2T": weight2[e].T.astype(BF16),
        }
        for e in range(N_CORES)
    ]


_CACHE = {}


def _get_nc():
    if "nc" not in _CACHE:
        _CACHE["nc"] = build_nc()
    return _CACHE["nc"]


def kernel(x, weight1, weight2):
    x = np.asarray(x, dtype=np.float32)
    weight1 = np.asarray(weight1, dtype=np.float32)
    weight2 = np.asarray(weight2, dtype=np.float32)
    assert x.shape == (N_CORES, TOK, DM)
    assert weight1.shape == (N_CORES, DH, DM)
    assert weight2.shape == (N_CORES, DM, DH)

    nc = _get_nc()
    in_maps = make_in_maps(x, weight1, weight2)
    res = run_bass_kernel_spmd(nc, in_maps, core_ids=list(range(N_CORES)))
    y = np.stack([res.results[e]["y"] for e in range(N_CORES)], axis=0)
    return y.reshape(1, N_CORES, TOK, DM)


# revision 21
# speedup vs baseline: 1.0095x; 1.0095x over previous
"""Expert-parallel MoE FFN kernel for Trainium2 (8 NeuronCores).

Problem: y[e] = relu(x[e] @ w1[e].T) @ w2[e].T for 8 experts.
Sharding: expert-parallel — expert e runs entirely on core e; no
cross-core communication.

Host-side prep (part of the sharding step): each expert's x, w1, w2 are
transposed to the layouts the PE array consumes (contraction dim on
partitions) and cast to bf16 (rel err ~3.1e-3 end to end). That removes
all 640 on-device PE transposes (which cost ~120us of tensor-engine
time via unhidden LDWEIGHTS) and cuts DMA-in from 40MB fp32 to 20MB.

Per-core dataflow (xT:[1024,2048], w1T:[1024,4096], w2T:[4096,1024]):
  w1T and w2T live in SBUF for the whole kernel (64 KiB/partition each).
  For each 512-token block:
    GEMM1: hT[dh_tile, tok] accumulated in PSUM over the 8 d_model
           tiles; fused ReLU + bf16 cast on eviction into a resident
           hT[128, 32, 512] tile. No DRAM spill.
    GEMM2: y[tok_tile, dm] accumulated in PSUM over the 32 d_hidden
           tiles against resident w2T; evicted fp32 and DMA'd out.
  All matmuls stream 512 columns (213ns, max: matmul out is capped at
  one 2KB PSUM bank) which hides every bf16 LDWEIGHTS (~97ns); the
  tensor engine does only the 2048 essential GEMM matmuls, ~95% busy.

Scheduling notes (measured on hw):
  - dma_start costs ~650ns of issue time on the sync engine; all DMAs
    issue from sync in consumption order (x block 0 and the first w1
    chunk interleaved per kt so GEMM1 starts ~12us in). gpsimd-issued
    DMAs become *software* DMA and steal SBUF bandwidth from the PE
    (20% slower matmuls); scalar-issued DMAs collide with evictions.
  - 10 dummy matmuls on a zeroed tile ramp the PE out of its low
    p-state while the first DMAs land.
  - PSUM->SBUF evictions alternate scalar/vector so neither engine
    gates the matmul stream.
"""

import sys

if "/opt/trn_rl_repo" not in sys.path:
    sys.path.insert(0, "/opt/trn_rl_repo")

import ml_dtypes
import numpy as np

import concourse.bass as bass  # noqa: F401
import concourse.mybir as mybir
from concourse import bacc
from concourse.bass_utils import run_bass_kernel_spmd
from concourse.tile import TileContext

P = 128
TOK = 2048
DM = 1024
DH = 4096
N_CORES = 8

KD = DM // P  # 8 d_model tiles (GEMM1 contraction)
KH = DH // P  # 32 d_hidden tiles (GEMM2 contraction)
TB = 512  # token block
NTB = TOK // TB  # 4 token blocks
MPB = TB // P  # 4 token tiles per block
QW = DM  # w1 load chunk (dh columns per DMA)

BF16 = ml_dtypes.bfloat16


def build_nc():
    f32 = mybir.dt.float32
    bf = mybir.dt.bfloat16
    nc = bacc.Bacc("TRN2", target_bir_lowering=False, debug=False)
    xT = nc.dram_tensor("xT", [DM, TOK], bf, kind="ExternalInput")
    w1T = nc.dram_tensor("w1T", [DM, DH], bf, kind="ExternalInput")
    w2T = nc.dram_tensor("w2T", [DH, DM], bf, kind="ExternalInput")
    y = nc.dram_tensor("y", [TOK, DM], f32, kind="ExternalOutput")

    relu = mybir.ActivationFunctionType.Relu
    copyf = mybir.ActivationFunctionType.Copy

    with TileContext(nc) as tc:
        with (
            tc.tile_pool(name="w1res", bufs=1) as w1p,
            tc.tile_pool(name="w2res", bufs=1) as w2p,
            tc.tile_pool(name="xt", bufs=NTB) as xp,
            tc.tile_pool(name="ht", bufs=1) as hp,
            tc.tile_pool(name="ys", bufs=4) as yp,
            tc.tile_pool(name="mm", bufs=8, space="PSUM") as mmp,
        ):
            w1t = w1p.tile([P, KD, DH], bf, name="w1t")
            w2t = w2p.tile([P, KH, DM], bf, name="w2t")

            # Startup: interleave first x block and first w1 chunk per kt
            # so GEMM1's first accumulation group can start as soon as the
            # kt=0 pair lands (DMA issue on the sync engine costs ~630ns
            # per dma_start, so issue order is arrival order).
            # All DMAs issue from the sync engine: its dma_start is the
            # hardware-queue path. (gpsimd.dma_start turns into software
            # dynamic DMA, which steals SBUF bandwidth from the PE and
            # slows every matmul ~20%.) Issue order is consumption order;
            # the first x block and first w1 chunk interleave per kt so
            # GEMM1's first group starts as soon as the kt=0 pair lands.
            xb = [xp.tile([P, KD, TB], bf, name="xt") for _ in range(NTB)]
            for kt in range(KD):
                nc.sync.dma_start(
                    xb[0][:, kt, :], xT[kt * P : (kt + 1) * P, 0:TB]
                )
                nc.sync.dma_start(
                    w1t[:, kt, 0:QW], w1T[kt * P : (kt + 1) * P, 0:QW]
                )
            for q in range(1, DH // QW):
                for kt in range(KD):
                    nc.sync.dma_start(
                        w1t[:, kt, q * QW : (q + 1) * QW],
                        w1T[kt * P : (kt + 1) * P, q * QW : (q + 1) * QW],
                    )
            for ht in range(KH):
                nc.sync.dma_start(w2t[:, ht, :], w2T[ht * P : (ht + 1) * P, :])
            for tb in range(1, NTB):
                for kt in range(KD):
                    nc.sync.dma_start(
                        xb[tb][:, kt, :],
                        xT[kt * P : (kt + 1) * P, tb * TB : (tb + 1) * TB],
                    )

            # PE warmup: dummy matmuls with no DMA dependency fill the
            # ~6us window while the first x/w1 chunks land, ramping the
            # tensor engine out of its low p-state before real work.
            wu = w1p.tile([P, TB], bf, name="wu")
            nc.vector.memset(wu[:], 0.0)
            for _ in range(10):
                pw = mmp.tile([P, TB], f32, tag="ps", name="psw")
                nc.tensor.matmul(pw[:], wu[:, 0:P], wu[:], start=True, stop=True)

            ncopy = [0]  # alternate PSUM->SBUF eviction engine

            def evict(dst, src, do_relu):
                ncopy[0] += 1
                if ncopy[0] % 2 == 0:
                    if do_relu:
                        nc.vector.tensor_scalar_max(dst, src, 0.0)
                    else:
                        nc.vector.tensor_copy(dst, src)
                else:
                    nc.scalar.activation(dst, src, relu if do_relu else copyf)

            for tb in range(NTB):
                hT = hp.tile([P, KH, TB], bf, name="hT")
                # GEMM1: hT[ht, tok] = relu(sum_kt w1T[kt,ht].T @ xT[kt,tok])
                for ht in range(KH):
                    ps = mmp.tile([P, TB], f32, tag="ps", name="ps1")
                    for kt in range(KD):
                        nc.tensor.matmul(
                            ps[:],
                            w1t[:, kt, ht * P : (ht + 1) * P],
                            xb[tb][:, kt, :],
                            start=(kt == 0),
                            stop=(kt == KD - 1),
                        )
                    evict(hT[:, ht, :], ps[:], True)
                # GEMM2: y[mt, dm] = sum_ht hT[ht, mt].T @ w2T[ht, dm]
                for mt in range(MPB):
                    for db in range(2):
                        # split the kernel's very last group in two so the
                        # final evict+DMA tail after the last matmul is half
                        # as long (matmul out is capped at 512 = 1 PSUM bank)
                        last = tb == NTB - 1 and mt == MPB - 1 and db == 1
                        for c0, cw in ((0, 256), (256, 256)) if last else ((0, TB),):
                            ps = mmp.tile([P, cw], f32, tag="ps", name="ps2")
                            col = db * TB + c0
                            for ht in range(KH):
                                nc.tensor.matmul(
                                    ps[:],
                                    hT[:, ht, mt * P : (mt + 1) * P],
                                    w2t[:, ht, col : col + cw],
                                    start=(ht == 0),
                                    stop=(ht == KH - 1),
                                )
                            ys = yp.tile([P, cw], f32, name="ys")
                            evict(ys[:], ps[:], False)
                            row = tb * TB + mt * P
                            nc.sync.dma_start(y[row : row + P, col : col + cw], ys[:])
    nc.compile()
    return nc


def make_in_maps(x, weight1, weight2):
    return [
        {
            "xT": x[e].T.astype(BF16),
            "w1T": weight1[e].T.astype(BF16),
            "w2T": weight2[e].T.astype(BF16),
        }
        for e in range(N_CORES)
    ]


_CACHE = {}


def _get_nc():
    if "nc" not in _CACHE:
        _CACHE["nc"] = build_nc()
    return _CACHE["nc"]


def kernel(x, weight1, weight2):
    x = np.asarray(x, dtype=np.float32)
    weight1 = np.asarray(weight1, dtype=np.float32)
    weight2 = np.asarray(weight2, dtype=np.float32)
    assert x.shape == (N_CORES, TOK, DM)
    assert weight1.shape == (N_CORES, DH, DM)
    assert weight2.shape == (N_CORES, DM, DH)

    nc = _get_nc()
    in_maps = make_in_maps(x, weight1, weight2)
    res = run_bass_kernel_spmd(nc, in_maps, core_ids=list(range(N_CORES)))
    y = np.stack([res.results[e]["y"] for e in range(N_CORES)], axis=0)
    return y.reshape(1, N_CORES, TOK, DM)


# revision 22
# speedup vs baseline: 1.0106x; 1.0011x over previous
"""Expert-parallel MoE FFN kernel for Trainium2 (8 NeuronCores).

Problem: y[e] = relu(x[e] @ w1[e].T) @ w2[e].T for 8 experts.
Sharding: expert-parallel — expert e runs entirely on core e; no
cross-core communication.

Host-side prep (part of the sharding step): each expert's x, w1, w2 are
transposed to the layouts the PE array consumes (contraction dim on
partitions) and cast to bf16 (rel err ~3.1e-3 end to end). That removes
all 640 on-device PE transposes (which cost ~120us of tensor-engine
time via unhidden LDWEIGHTS) and cuts DMA-in from 40MB fp32 to 20MB.

Per-core dataflow (xT:[1024,2048], w1T:[1024,4096], w2T:[4096,1024]):
  w1T and w2T live in SBUF for the whole kernel (64 KiB/partition each).
  For each 512-token block:
    GEMM1: hT[dh_tile, tok] accumulated in PSUM over the 8 d_model
           tiles; fused ReLU + bf16 cast on eviction into a resident
           hT[128, 32, 512] tile. No DRAM spill.
    GEMM2: y[tok_tile, dm] accumulated in PSUM over the 32 d_hidden
           tiles against resident w2T; evicted fp32 and DMA'd out.
  All matmuls stream 512 columns (213ns, max: matmul out is capped at
  one 2KB PSUM bank) which hides every bf16 LDWEIGHTS (~97ns); the
  tensor engine does only the 2048 essential GEMM matmuls, ~95% busy.

Scheduling notes (measured on hw):
  - dma_start costs ~650ns of issue time on the sync engine; all DMAs
    issue from sync in consumption order (x block 0 and the first w1
    chunk interleaved per kt so GEMM1 starts ~12us in). gpsimd-issued
    DMAs become *software* DMA and steal SBUF bandwidth from the PE
    (20% slower matmuls); scalar-issued DMAs collide with evictions.
  - 10 dummy matmuls on a zeroed tile ramp the PE out of its low
    p-state while the first DMAs land.
  - PSUM->SBUF evictions alternate scalar/vector so neither engine
    gates the matmul stream.
"""

import sys

if "/opt/trn_rl_repo" not in sys.path:
    sys.path.insert(0, "/opt/trn_rl_repo")

import ml_dtypes
import numpy as np

import concourse.bass as bass  # noqa: F401
import concourse.mybir as mybir
from concourse import bacc
from concourse.bass_utils import run_bass_kernel_spmd
from concourse.tile import TileContext

P = 128
TOK = 2048
DM = 1024
DH = 4096
N_CORES = 8

KD = DM // P  # 8 d_model tiles (GEMM1 contraction)
KH = DH // P  # 32 d_hidden tiles (GEMM2 contraction)
TB = 512  # token block
NTB = TOK // TB  # 4 token blocks
MPB = TB // P  # 4 token tiles per block
QW = DM  # w1 load chunk (dh columns per DMA)

BF16 = ml_dtypes.bfloat16


def build_nc():
    f32 = mybir.dt.float32
    bf = mybir.dt.bfloat16
    nc = bacc.Bacc("TRN2", target_bir_lowering=False, debug=False)
    xT = nc.dram_tensor("xT", [DM, TOK], bf, kind="ExternalInput")
    w1T = nc.dram_tensor("w1T", [DM, DH], bf, kind="ExternalInput")
    w2T = nc.dram_tensor("w2T", [DH, DM], bf, kind="ExternalInput")
    y = nc.dram_tensor("y", [TOK, DM], f32, kind="ExternalOutput")

    relu = mybir.ActivationFunctionType.Relu
    copyf = mybir.ActivationFunctionType.Copy

    with TileContext(nc) as tc:
        with (
            tc.tile_pool(name="w1res", bufs=1) as w1p,
            tc.tile_pool(name="w2res", bufs=1) as w2p,
            tc.tile_pool(name="xt", bufs=NTB) as xp,
            tc.tile_pool(name="ht", bufs=1) as hp,
            tc.tile_pool(name="ys", bufs=4) as yp,
            tc.tile_pool(name="mm", bufs=8, space="PSUM") as mmp,
        ):
            w1t = w1p.tile([P, KD, DH], bf, name="w1t")
            w2t = w2p.tile([P, KH, DM], bf, name="w2t")

            # All DMAs issue from the sync engine: its dma_start is the
            # hardware-queue path (gpsimd's becomes software dynamic DMA,
            # which steals SBUF bandwidth from the PE; scalar's collides
            # with evictions). Issue order is consumption order; the first
            # x block and first w1 chunk interleave per kt so GEMM1's
            # first group starts as soon as the kt=0 pair lands.
            xb = [xp.tile([P, KD, TB], bf, name="xt") for _ in range(NTB)]
            for kt in range(KD):
                nc.sync.dma_start(
                    xb[0][:, kt, :], xT[kt * P : (kt + 1) * P, 0:TB]
                )
                nc.sync.dma_start(
                    w1t[:, kt, 0:QW], w1T[kt * P : (kt + 1) * P, 0:QW]
                )
            for q in range(1, DH // QW):
                for kt in range(KD):
                    nc.sync.dma_start(
                        w1t[:, kt, q * QW : (q + 1) * QW],
                        w1T[kt * P : (kt + 1) * P, q * QW : (q + 1) * QW],
                    )
            for ht in range(KH):
                nc.sync.dma_start(w2t[:, ht, :], w2T[ht * P : (ht + 1) * P, :])
            for tb in range(1, NTB):
                for kt in range(KD):
                    nc.sync.dma_start(
                        xb[tb][:, kt, :],
                        xT[kt * P : (kt + 1) * P, tb * TB : (tb + 1) * TB],
                    )

            # PE warmup: dummy matmuls with no DMA dependency fill the
            # ~6us window while the first x/w1 chunks land, ramping the
            # tensor engine out of its low p-state before real work.
            wu = w1p.tile([P, TB], bf, name="wu")
            nc.vector.memset(wu[:], 0.0)
            for _ in range(10):
                pw = mmp.tile([P, TB], f32, tag="ps", name="psw")
                nc.tensor.matmul(pw[:], wu[:, 0:P], wu[:], start=True, stop=True)

            ncopy = [0]  # alternate PSUM->SBUF eviction engine

            def evict(dst, src, do_relu):
                ncopy[0] += 1
                if ncopy[0] % 2 == 0:
                    if do_relu:
                        nc.vector.tensor_scalar_max(dst, src, 0.0)
                    else:
                        nc.vector.tensor_copy(dst, src)
                else:
                    nc.scalar.activation(dst, src, relu if do_relu else copyf)

            for tb in range(NTB):
                hT = hp.tile([P, KH, TB], bf, name="hT")
                # GEMM1: hT[ht, tok] = relu(sum_kt w1T[kt,ht].T @ xT[kt,tok])
                for ht in range(KH):
                    ps = mmp.tile([P, TB], f32, tag="ps", name="ps1")
                    for kt in range(KD):
                        nc.tensor.matmul(
                            ps[:],
                            w1t[:, kt, ht * P : (ht + 1) * P],
                            xb[tb][:, kt, :],
                            start=(kt == 0),
                            stop=(kt == KD - 1),
                        )
                    evict(hT[:, ht, :], ps[:], True)
                # GEMM2: y[mt, dm] = sum_ht hT[ht, mt].T @ w2T[ht, dm]
                for mt in range(MPB):
                    for db in range(2):
                        # split the kernel's very last group in two so the
                        # final evict+DMA tail after the last matmul is half
                        # as long (matmul out is capped at 512 = 1 PSUM bank)
                        last = tb == NTB - 1 and mt == MPB - 1 and db == 1
                        for c0, cw in ((0, 256), (256, 256)) if last else ((0, TB),):
                            ps = mmp.tile([P, cw], f32, tag="ps", name="ps2")
                            col = db * TB + c0
                            for ht in range(KH):
                                nc.tensor.matmul(
                                    ps[:],
                                    hT[:, ht, mt * P : (mt + 1) * P],
                                    w2t[:, ht, col : col + cw],
                                    start=(ht == 0),
                                    stop=(ht == KH - 1),
                                )
                            ys = yp.tile([P, cw], f32, name="ys")
                            evict(ys[:], ps[:], False)
                            row = tb * TB + mt * P
                            nc.sync.dma_start(y[row : row + P, col : col + cw], ys[:])
    nc.compile()
    return nc


def make_in_maps(x, weight1, weight2):
    return [
        {
            "xT": x[e].T.astype(BF16),
            "w1T": weight1[e].T.astype(BF16),
            "w2T": weight2[e].T.astype(BF16),
        }
        for e in range(N_CORES)
    ]


_CACHE = {}


def _get_nc():
    if "nc" not in _CACHE:
        _CACHE["nc"] = build_nc()
    return _CACHE["nc"]


def kernel(x, weight1, weight2):
    x = np.asarray(x, dtype=np.float32)
    weight1 = np.asarray(weight1, dtype=np.float32)
    weight2 = np.asarray(weight2, dtype=np.float32)
    assert x.shape == (N_CORES, TOK, DM)
    assert weight1.shape == (N_CORES, DH, DM)
    assert weight2.shape == (N_CORES, DM, DH)

    nc = _get_nc()
    in_maps = make_in_maps(x, weight1, weight2)
    res = run_bass_kernel_spmd(nc, in_maps, core_ids=list(range(N_CORES)))
    y = np.stack([res.results[e]["y"] for e in range(N_CORES)], axis=0)
    return y.reshape(1, N_CORES, TOK, DM)
